# revision 27
# baseline (speedup 1.0000x reference)
"""Trainium2 Bass kernel for CustomMinkowskiLayerNorm.

Math (matches the jax reference):
    counts[b]  = #points with batch_indices == b           (clamped >= 1)
    mean[b,c]  = sum_{i in b} x[i,c] / counts[b]
    var[b,c]   = sum_{i in b} (x[i,c]-mean)^2 / counts[b]  (= E[x^2]-mean^2)
    out[i,c]   = (x[i,c]-mean[b_i,c]) / sqrt(var[b_i,c]+eps) * gamma[c] + beta[c]

Sharding: batch_indices is sorted and BATCH == n_cores == 8, so each core owns
exactly one batch segment -> all segment reductions are core-local, no
collectives. The host splits at segment boundaries (searchsorted), transposes
each segment to channel-major layout and zero-pads to a fixed shape:

    xt[p, f], p in [0,128): partition p < 64  = channel p,  points [0, F_HALF)
                            partition p >= 64 = channel p-64, points [F_HALF, 2*F_HALF)

Channel-major layout makes the per-channel segment reduction a free-dim
reduction and the normalization a single per-partition affine op.

fp16 I/O: the kernel is HBM-bandwidth-bound (per-core HBM limit ~358 GB/s,
shared read+write; measured ~350 sustained). The rel-err budget (2e-2) is
~40x above fp16 quantization error (~5e-4), so the host casts features to
fp16 and the device moves half the bytes each direction; the host upcasts
the fp16 result to f32. Statistics accumulate in f32 on device, so mean/var
carry only the input quantization error.

Measured op rates (HW, per col of 128-partition fp16 data):
    DMA arrival          0.73 ns   bn_stats (DVE)        1.32 ns (sum+sumsq)
    ACT Copy|Square+acc  0.72 ns/op + 1.27 us/op fixed
    tensor_scalar 4x     0.28 ns   (in-place fp16 affine)
    tensor_scalar+accum  1.06 ns   (demoted to 1x CACHE_REDUCE - not used)

Device program (per core, identical SPMD):
  pass 1: loads stream on the sync HWDGE ring; tile sizes ramp 2K->8K cols
          (4..16KB lines) so stats start early, and shrink back to 2K at the
          end so the last tile's stats drain fast. Per-tile stats are
          list-scheduled onto DVE (bn_stats per 512-col chunk) or ScalarE
          (Copy+accum_out / Square+accum_out pair) using the measured rates;
          combined engine rate slightly beats the DMA stream.
  stats:  DVE groups -> split bn_aggr (bulk early, last DVE tile late) ->
          raw (sum, sumsq); ACT accumulators reduced and added; fold
          partitions p/p+64 and broadcast with one TensorE matmul against a
          0/1 fold matrix; mean/E[x^2] via 1/count; rstd = 1/sqrt(var+eps)
          via ACT sqrt + DVE reciprocal + 1 Newton step; s = gamma*rstd,
          t = beta - mean*s.
  pass 2: x = x*s + t in place (DVE tensor_scalar, 4x fp16 mode), stores on
          the scalar HWDGE ring, smallest tiles first.
"""

import os
import sys

for _p in ("/opt/trn_rl_repo", "/root/.axon_site/_ro/trn_rl_repo"):
    if os.path.isdir(_p) and _p not in sys.path:
        sys.path.append(_p)

from contextlib import ExitStack

import numpy as np

import concourse.bacc as bacc
import concourse.tile as tile
from concourse import mybir
from concourse._compat import with_exitstack
from concourse.bass_utils import run_bass_kernel_spmd

F32 = mybir.dt.float32
F16 = mybir.dt.float16

N = 1_000_000
C = 64
BATCH = 8
EPS = 1e-5

P = 128            # SBUF partitions
GRAN = 2048        # f_half granularity (cols)
MIDF = 8192        # steady-state tile free size (16 KB fp16 per line)
BN_F = 512         # bn_stats free-dim max

# measured constants for the pass-1 list scheduler (ns; per fp16 column of
# 128 partitions where applicable)
_RAMP_NS = 6000.0   # NEFF start -> first DMA bytes
_DMA_NS = 0.61      # load stream rate (~420 GB/s steady)
_DVE_NS = 1.32      # bn_stats per column (sum+sumsq)
_ACT_NS = 0.715     # ACT activation linear per column per op
_ACT_FIX = 1270.0   # ACT per-op fixed cost (incl. accumulator read)
_DVE_RDY = 5000.0   # DVE free after preamble
_ACT_RDY = 12600.0  # ACT free after table warm-up loads

_mult = mybir.AluOpType.mult
_add = mybir.AluOpType.add

_AF = mybir.ActivationFunctionType


def _tile_sizes(f_half: int):
    """Tile free-sizes: ramp up 2K,4K -> 8K steady -> remainder -> 2K tail.
    Early small tiles let engine stats start ~7us in; the small last tile
    keeps the last-load -> stats-done critical path short."""
    if f_half <= 4 * GRAN:
        return [GRAN] * (f_half // GRAN)
    head = [GRAN, 2 * GRAN]
    tail = [GRAN]
    rem = f_half - 4 * GRAN
    mid = [MIDF] * (rem // MIDF)
    rem -= (rem // MIDF) * MIDF
    fill = []
    while rem > 0:
        s = GRAN
        while s * 2 <= rem:
            s *= 2
        fill.append(s)
        rem -= s
    return head + mid + fill + tail


def _schedule_stats(sizes):
    """Exhaustive min-makespan split of each tile's columns between DVE
    (bn_stats on the head slice) and ACT (Copy+Square accum pair on the tail
    slice), simulating DMA arrivals and engine availability with measured
    rates. Returns act_width per tile (multiple of 2048, 0 = all DVE)."""
    import itertools

    nt = len(sizes)
    arr = []
    t = _RAMP_NS
    for s in sizes:
        t += s * _DMA_NS
        arr.append(t)
    opts = []
    for i, s in enumerate(sizes):
        if s >= 4096 and i < nt - 1:  # keep the last tile all-DVE (fast drain)
            opts.append(list(range(0, s + 1, 2048)))
        else:
            opts.append([0])
    best = None
    for ws in itertools.product(*opts):
        dve = _DVE_RDY
        act = _ACT_RDY
        for i, s in enumerate(sizes):
            w = ws[i]
            d = s - w
            if d:
                dve = max(arr[i], dve) + d * _DVE_NS
            if w:
                act = max(arr[i], act) + 2.0 * (w * _ACT_NS + _ACT_FIX)
        mk = max(dve, act)
        if best is None or mk < best[0]:
            best = (mk, ws)
    return list(best[1])


def _make_body(f_half: int):
    sizes = _tile_sizes(f_half)
    nt = len(sizes)
    offs = [0]
    for s in sizes:
        offs.append(offs[-1] + s)
    act_w = _schedule_stats(sizes)
    dve_w = [sizes[t] - act_w[t] for t in range(nt)]
    act_tiles = [t for t in range(nt) if act_w[t] > 0]
    n_dve_cols = sum(dve_w)
    n_act_cols = sum(act_w)
    ngrp = n_dve_cols // BN_F

    @with_exitstack
    def _body(ctx: ExitStack, tc: tile.TileContext,
              out_ap, xt_ap, invn_ap, gcol_ap, bcol_ap, foldm_ap):
        nc = tc.nc

        cache = ctx.enter_context(tc.tile_pool(name="cache", bufs=1))
        small = ctx.enter_context(tc.tile_pool(name="small", bufs=1))
        psum = ctx.enter_context(tc.tile_pool(name="psum", bufs=1, space="PSUM"))

        # Small inputs ride the scalar HWDGE ring (sync ring streams tiles).
        invn_sb = small.tile([P, 1], F32, tag="invn")
        gcol_sb = small.tile([P, 1], F32, tag="gcol")
        bcol_sb = small.tile([P, 1], F32, tag="bcol")
        foldm_sb = small.tile([P, P], F32, tag="foldm")
        nc.scalar.dma_start(out=invn_sb, in_=invn_ap)
        nc.scalar.dma_start(out=gcol_sb, in_=gcol_ap)
        nc.scalar.dma_start(out=bcol_sb, in_=bcol_ap)
        nc.scalar.dma_start(out=foldm_sb, in_=foldm_ap)

        # Pre-load the ACT table set (Square/Sqrt) so pass-1 ACT ops and the
        # stats chain don't stall on ACT_TABLE_LOAD.
        warm = small.tile([P, 1], F32, tag="warm")
        nc.vector.memset(warm, 1.0)
        nc.scalar.activation(out=warm, in_=warm, func=_AF.Square)
        nc.scalar.activation(out=warm, in_=warm, func=_AF.Sqrt)

        stats = small.tile([P, max(ngrp, 1), 6], F32, tag="stats")
        n_act = len(act_tiles)
        acc_a = small.tile([P, max(n_act, 1), 2], F32, tag="acc_a")
        scr_a = small.tile([P, MIDF], F16, tag="scr_a")

        # ---- pass 1: stream tiles on sync ring; stats per schedule ----
        # Each tile's head slice goes to DVE bn_stats; its tail slice (act_w)
        # to an ACT Copy+accum / Square+accum pair. Split bn_aggr: the bulk
        # (all groups except the last DVE-carrying tile's) is emitted right
        # after that tile's chunks so it aggregates while tail tiles load.
        last_dve = max(t for t in range(nt) if dve_w[t] > 0)
        gb = dve_w[last_dve] // BN_F
        ga = ngrp - gb
        mva = small.tile([P, 2], F32, tag="mva")
        xt_all = cache.tile([P, f_half], F16, tag="c")
        tiles = []
        grp = 0
        ai = 0
        aggr_done = False
        for t in range(nt):
            sl = slice(offs[t], offs[t] + sizes[t])
            xt = xt_all[:, sl]
            tiles.append(xt)
            nc.sync.dma_start(out=xt, in_=xt_ap[:, sl])
            for j in range(dve_w[t] // BN_F):
                nc.vector.bn_stats(
                    out=stats[:, grp, :],
                    in_=xt[:, j * BN_F : (j + 1) * BN_F],
                )
                grp += 1
            if ga > 0 and grp == ga and not aggr_done:
                nc.vector.bn_aggr(out=mva, in_=stats[:, :ga, :])
                aggr_done = True
            w = act_w[t]
            if w > 0:
                asl = xt[:, sizes[t] - w : sizes[t]]
                nc.scalar.activation(out=scr_a[:, :w], in_=asl, func=_AF.Copy,
                                     accum_out=acc_a[:, ai, 0:1])
                nc.scalar.activation(out=scr_a[:, :w], in_=asl, func=_AF.Square,
                                     accum_out=acc_a[:, ai, 1:2])
                ai += 1

        # ---- aggregate stats ----
        sums = small.tile([P, 2], F32, tag="sums")

        def raw_sums(dst, mv, n):
            # mv = (mean, var) over n cols -> dst = (sum, sumsq) = n*(mean,
            # var+mean^2). STT fuses var+mean^2 in place, then one TS scales.
            nc.vector.scalar_tensor_tensor(out=mv[:, 1:2], in0=mv[:, 0:1],
                                           scalar=mv[:, 0:1], in1=mv[:, 1:2],
                                           op0=_mult, op1=_add)
            nc.vector.tensor_scalar_mul(out=dst, in0=mv, scalar1=float(n))

        parts = []
        if ga > 0:
            sums_a = small.tile([P, 2], F32, tag="sums_a")
            raw_sums(sums_a, mva, ga * BN_F)
            parts.append(sums_a)
        if gb > 0:
            mvb = small.tile([P, 2], F32, tag="mvb")
            nc.vector.bn_aggr(out=mvb, in_=stats[:, ga:ngrp, :])
            sums_b = small.tile([P, 2], F32, tag="sums_b")
            raw_sums(sums_b, mvb, gb * BN_F)
            parts.append(sums_b)
        if n_act:
            sums_c = small.tile([P, 2], F32, tag="sums_c")
            aview = acc_a.rearrange("p t c -> p c t")
            nc.vector.reduce_sum(out=sums_c, in_=aview,
                                 axis=mybir.AxisListType.X)
            parts.append(sums_c)
        acc = parts[0]
        for pt in parts[1:]:
            nc.vector.tensor_add(out=sums, in0=acc, in1=pt)
            acc = sums
        if acc is not sums:
            nc.vector.tensor_copy(out=sums, in_=acc)

        # ---- fold halves + broadcast: tot[p] = sums[p%64] + sums[p%64+64] ----
        ptot = psum.tile([P, 2], F32, tag="pt")
        nc.tensor.matmul(out=ptot, lhsT=foldm_sb, rhs=sums,
                         start=True, stop=True)
        tot = small.tile([P, 2], F32, tag="tot")
        nc.vector.tensor_copy(out=tot, in_=ptot)

        # ---- per-channel coefficients ----
        mm = small.tile([P, 2], F32, tag="mm")      # (mean, E[x^2])
        nc.vector.tensor_scalar_mul(out=mm, in0=tot, scalar1=invn_sb[:, 0:1])
        var = small.tile([P, 1], F32, tag="var")
        nc.vector.tensor_mul(out=var, in0=mm[:, 0:1], in1=mm[:, 0:1])
        nc.vector.tensor_sub(out=var, in0=mm[:, 1:2], in1=var)
        v = small.tile([P, 1], F32, tag="v")
        nc.vector.tensor_scalar(out=v, in0=var, scalar1=0.0, scalar2=EPS,
                                op0=mybir.AluOpType.max, op1=_add)
        r = small.tile([P, 1], F32, tag="r")
        nc.scalar.activation(out=r, in_=v, func=_AF.Sqrt)
        nc.vector.reciprocal(out=r, in_=r)
        # no Newton refinement: ACT sqrt table error (~1e-3) is far below the
        # 2e-2 rel-err budget and the chain sits on the critical path.
        s_col = small.tile([P, 1], F32, tag="s_col")
        nc.vector.tensor_mul(out=s_col, in0=r, in1=gcol_sb)
        t_col = small.tile([P, 1], F32, tag="t_col")
        nc.vector.tensor_mul(out=t_col, in0=mm[:, 0:1], in1=s_col)
        nc.vector.tensor_sub(out=t_col, in0=bcol_sb, in1=t_col)

        # ---- pass 2: x = x*s + t in place (DVE 4x fp16), stores on the
        # scalar HWDGE ring. Smallest tile first (store stream starts on the
        # shortest affine); one small tile last (the end-of-kernel completion
        # receipt then covers a short transfer).
        order = sorted(range(nt), key=lambda t: (sizes[t], -t))
        for t in order:
            sl = slice(offs[t], offs[t] + sizes[t])
            xt = tiles[t]
            nc.vector.tensor_scalar(out=xt, in0=xt, scalar1=s_col[:, 0:1],
                                    scalar2=t_col[:, 0:1],
                                    op0=_mult, op1=_add)
            nc.scalar.dma_start(out=out_ap[:, sl], in_=xt)

    return _body


_NC_CACHE = {}


def _build_program(f_half: int):
    if f_half in _NC_CACHE:
        return _NC_CACHE[f_half]
    nc = bacc.Bacc("TRN2", target_bir_lowering=False, debug=False,
                   num_devices=BATCH)
    xt = nc.dram_tensor("xt", [P, f_half], F16, kind="ExternalInput").ap()
    invn = nc.dram_tensor("invn", [P, 1], F32, kind="ExternalInput").ap()
    gcol = nc.dram_tensor("gcol", [P, 1], F32, kind="ExternalInput").ap()
    bcol = nc.dram_tensor("bcol", [P, 1], F32, kind="ExternalInput").ap()
    foldm = nc.dram_tensor("foldm", [P, P], F32, kind="ExternalInput").ap()
    out = nc.dram_tensor("out", [P, f_half], F16, kind="ExternalOutput").ap()
    with tile.TileContext(nc) as tc:
        _make_body(f_half)(tc, out, xt, invn, gcol, bcol, foldm)
    nc.compile()
    _NC_CACHE[f_half] = nc
    return nc


def _prepare(features, batch_indices, gamma, beta):
    features = np.asarray(features, dtype=np.float32)
    batch_indices = np.asarray(batch_indices, dtype=np.int32)
    gamma = np.asarray(gamma, dtype=np.float32)
    beta = np.asarray(beta, dtype=np.float32)

    bounds = np.searchsorted(batch_indices, np.arange(BATCH + 1), side="left")
    cnts = np.diff(bounds)
    # fixed SPMD shape: half-row length, padded to a multiple of GRAN
    f_half = max(int(-(-int(cnts.max()) // 2 // GRAN) * GRAN), GRAN)

    gcol = np.concatenate([gamma, gamma]).reshape(P, 1).astype(np.float32)
    bcol = np.concatenate([beta, beta]).reshape(P, 1).astype(np.float32)
    k = np.arange(P)
    foldm = (k[:, None] % C == k[None, :] % C).astype(np.float32)

    feats16 = features.astype(np.float16)
    in_maps = []
    for b in range(BATCH):
        s, e = int(bounds[b]), int(bounds[b + 1])
        cnt = e - s
        xt = np.zeros((P, f_half), dtype=np.float16)
        n1 = min(cnt, f_half)
        if n1 > 0:
            xt[0:C, :n1] = feats16[s : s + n1].T
        if cnt > f_half:
            xt[C:P, : cnt - f_half] = feats16[s + f_half : e].T
        in_maps.append({
            "xt": xt,
            "invn": np.full((P, 1), 1.0 / max(cnt, 1), dtype=np.float32),
            "gcol": gcol,
            "bcol": bcol,
            "foldm": foldm,
        })
    return in_maps, bounds, f_half


def _assemble(results, bounds, f_half):
    out = np.empty((N, C), dtype=np.float32)
    for b in range(BATCH):
        s, e = int(bounds[b]), int(bounds[b + 1])
        cnt = e - s
        if cnt == 0:
            continue
        ot = results[b]["out"].astype(np.float32)
        n1 = min(cnt, f_half)
        out[s : s + n1] = ot[0:C, :n1].T
        if cnt > f_half:
            out[s + f_half : e] = ot[C:P, : cnt - f_half].T
    return out


def run_with_results(features, batch_indices, gamma, beta, **run_kwargs):
    in_maps, bounds, f_half = _prepare(features, batch_indices, gamma, beta)
    nc = _build_program(f_half)
    res = run_bass_kernel_spmd(nc, in_maps, core_ids=list(range(BATCH)),
                               **run_kwargs)
    return _assemble(res.results, bounds, f_half), res


def kernel(features, batch_indices, gamma, beta):
    out, _ = run_with_results(features, batch_indices, gamma, beta)
    return out
